# revision 1
# baseline (speedup 1.0000x reference)
"""Trainium2 Bass kernel for nn_Luban7_29609504539316 (BiLSTM + span pool + log_softmax).

Sharding (8 cores):
  - Direction-split scan: cores 0-3 run the FORWARD LSTM, cores 4-7 the BACKWARD
    LSTM (fed host-reversed tokens).  Core c handles batch group g = c % 4
    (batches g*8 .. g*8+8) for the scan.
  - Pair (c, c+4) exchanges hidden states via a 2-core AllGather; every core then
    runs the post-LSTM stages (lin1/lin2/LN/span-pool/label) for all 8 of its
    group's batches.  The global (axis-0) log_softmax max/sum are produced with
    AllReduce over {0,1,2,3} / {4,5,6,7} (each group covers all 32 batches).
  - Host takes the outputs of cores 0-3 (batch groups 0-3) and concatenates.

The program is identical on all cores (SPMD); direction and batch assignment
live entirely in the per-core input data (tokens, per-direction weights).
"""

import os
import sys

import numpy as np

for _p in ("/opt/trn_rl_repo",):
    if _p not in sys.path and os.path.isdir(_p):
        sys.path.insert(0, _p)

import concourse.bass as bass
import concourse.tile as tile
from concourse import bacc
from concourse import mybir
from concourse.bass_utils import run_bass_kernel_spmd

F32 = mybir.dt.float32
F32R = mybir.dt.float32r
I32 = mybir.dt.int32
AF = mybir.ActivationFunctionType
OP = mybir.AluOpType

# Problem dims (hardcoded per spec)
B, T, V, E, H, C, L = 32, 120, 32000, 256, 512, 20, 10
G4 = 4 * H  # 2048
LN_EPS = 1e-5
NCORES = 8
BL = 8            # batches per scan core
ROWS = BL * T     # 960
SROWS = BL * 1155  # spans rows per core = 9240

# static span table (matches reference loop order)
_begs, _lens = [], []
for _b in range(T):
    for _l in range(1, min(L, T - _b) + 1):
        _begs.append(_b)
        _lens.append(_l)
BEGS = np.asarray(_begs, np.int32)
LENS = np.asarray(_lens, np.int32)
S = len(_begs)  # 1155
assert S == 1155


def _mspanT() -> np.ndarray:
    """[T, S] span-mean pooling matrix (inv_len folded in)."""
    m = np.zeros((T, S), np.float32)
    for s in range(S):
        m[BEGS[s] : BEGS[s] + LENS[s], s] = 1.0 / LENS[s]
    return m


def _gather_idx() -> np.ndarray:
    """[2*ROWS, 1] reorder table: fwd rows identity, bwd rows t-reversed."""
    idx = np.empty(2 * ROWS, np.int32)
    idx[:ROWS] = np.arange(ROWS, dtype=np.int32)
    for b in range(BL):
        for t in range(T):
            idx[ROWS + b * T + t] = ROWS + b * T + (T - 1 - t)
    return idx[:, None]


def _r(ap):
    return ap.bitcast(F32R)


def build_program():
    max_stage = int(os.environ.get("KERNEL_MAX_STAGE", "99"))
    ablate = os.environ.get("KERNEL_SCAN_ABLATE", "none")
    scan_steps = int(os.environ.get("KERNEL_SCAN_STEPS", str(T)))
    nc = bacc.Bacc(trn_type="TRN2", num_devices=NCORES)

    # ---- I/O ----
    tok = nc.dram_tensor("tok", [ROWS, 1], I32, kind="ExternalInput")
    emb = nc.dram_tensor("emb", [V, E], F32, kind="ExternalInput")
    w_ihT = nc.dram_tensor("w_ihT", [E, G4], F32R, kind="ExternalInput")
    w_hhT = nc.dram_tensor("w_hhT", [H, G4], F32R, kind="ExternalInput")
    gbias = nc.dram_tensor("gbias", [1, G4], F32, kind="ExternalInput")
    lin1_wT = nc.dram_tensor("lin1_wT", [2 * H, H], F32R, kind="ExternalInput")
    lin1_b = nc.dram_tensor("lin1_b", [H, 1], F32, kind="ExternalInput")
    lin2_wT = nc.dram_tensor("lin2_wT", [H, H], F32R, kind="ExternalInput")
    lin2_b = nc.dram_tensor("lin2_b", [1, H], F32, kind="ExternalInput")
    ln_g = nc.dram_tensor("ln_g", [1, H], F32, kind="ExternalInput")
    ln_b = nc.dram_tensor("ln_b", [1, H], F32, kind="ExternalInput")
    label_w = nc.dram_tensor("label_w", [H, C], F32R, kind="ExternalInput")
    label_b = nc.dram_tensor("label_b", [C, 1], F32, kind="ExternalInput")
    out = nc.dram_tensor("out", [SROWS, C], F32, kind="ExternalOutput")

    # ---- inline constants (same on every core) ----
    ident_d = nc.inline_tensor(np.eye(128, dtype=np.float32), name="ident")
    mspanT_d = nc.inline_tensor(_mspanT(), name="mspanT")
    gidx_d = nc.inline_tensor(_gather_idx(), name="gidx")

    with tile.TileContext(nc) as tc:
        # long-lived pools (released at end of build)
        const_p = tc.alloc_tile_pool(name="const", bufs=1)
        whh_p = tc.alloc_tile_pool(name="whh", bufs=1)
        dram_p = tc.alloc_tile_pool(name="dram", bufs=1, space="DRAM")

        ident = const_p.tile([128, 128], F32)
        nc.sync.dma_start(out=ident[:], in_=ident_d[:, :])
        ident_r = const_p.tile([128, 128], F32R)
        nc.gpsimd.dma_start(out=ident_r[:], in_=ident_d[:, :])

        w_hhT_sb = [whh_p.tile([128, G4], F32R, tag=f"whh{k}", name=f"whh{k}") for k in range(4)]
        for k in range(4):
            nc.sync.dma_start(out=w_hhT_sb[k][:], in_=w_hhT[128 * k : 128 * (k + 1), :])

        xg_dram = dram_p.tile([T, BL, G4], F32R)
        hs_dram = dram_p.tile([ROWS, H], F32)
        hs_ag = dram_p.tile([2 * ROWS, H], F32)

        # ================= Stage 1: embedding gather + transpose =============
        with tc.tile_pool(name="s1", bufs=3) as s1p, \
             tc.tile_pool(name="s1ps", bufs=4, space="PSUM") as s1ps, \
             tc.tile_pool(name="xt", bufs=1) as xt_p, \
             tc.tile_pool(name="wih", bufs=1) as wih_p:

            xT = [xt_p.tile([128, ROWS], F32R, tag=f"xT{k}", name=f"xT{k}") for k in range(2)]
            w_ihT_sb = [wih_p.tile([128, G4], F32R, tag=f"wih{k}", name=f"wih{k}") for k in range(2)]
            gbias_sb = wih_p.tile([T, G4], F32)
            for k in range(2):
                nc.sync.dma_start(
                    out=w_ihT_sb[k][:], in_=w_ihT[128 * k : 128 * (k + 1), :]
                )
            nc.gpsimd.dma_start(
                out=gbias_sb[:], in_=gbias[:, :].to_broadcast([T, G4])
            )

            n_full = ROWS // 128          # 7
            tail = ROWS - n_full * 128    # 64
            for r in range(n_full + 1):
                rows = 128 if r < n_full else tail
                idx_sb = s1p.tile([128, 1], I32, tag="idx")
                x_sb = s1p.tile([128, E], F32, tag="x")
                nc.sync.dma_start(
                    out=idx_sb[:rows], in_=tok[r * 128 : r * 128 + rows, :]
                )
                nc.gpsimd.indirect_dma_start(
                    out=x_sb[:rows, :],
                    out_offset=None,
                    in_=emb[:, :],
                    in_offset=bass.IndirectOffsetOnAxis(ap=idx_sb[:rows, :1], axis=0),
                )
                for k in range(2):
                    pt = s1ps.tile([128, 128], F32, tag="pt")
                    nc.tensor.transpose(
                        out=pt[:, :rows],
                        in_=x_sb[:rows, 128 * k : 128 * (k + 1)],
                        identity=ident[:rows, :rows],
                    )
                    nc.vector.tensor_copy(
                        out=xT[k][:, r * 128 : r * 128 + rows], in_=pt[:, :rows]
                    )

            # ================= Stage 2: xg = x @ w_ihT + bias ================
            with tc.tile_pool(name="s2", bufs=3) as s2p, \
                 tc.tile_pool(name="s2ps", bufs=3, space="PSUM") as s2ps:
                for b in range(BL):
                    for n in range(4):
                        ps = s2ps.tile([T, 512], F32, tag="ps")
                        for k in range(2):
                            nc.tensor.matmul(
                                ps[:],
                                lhsT=xT[k][:, b * T : (b + 1) * T],
                                rhs=w_ihT_sb[k][:, 512 * n : 512 * (n + 1)],
                                start=(k == 0),
                                stop=(k == 1),
                            )
                        stg = s2p.tile([T, 512], F32R, tag="stg")
                        nc.vector.tensor_tensor(
                            out=stg[:],
                            in0=ps[:],
                            in1=gbias_sb[:, 512 * n : 512 * (n + 1)],
                            op=OP.add,
                        )
                        nc.sync.dma_start(
                            out=xg_dram[:, b, 512 * n : 512 * (n + 1)], in_=stg[:]
                        )

        # ================= Stage 3: LSTM scan (this core's direction) ========
        # Layout: all 4 gates in ONE psum tile pg_all[128,512] at partition
        # offsets 32n (col-tiled matmuls); batch padded 8->32 with zeros so
        # every read row is defined.  The xg contribution is accumulated by an
        # extra identity-matmul per gate (no DVE adds).  h lives in h_pad
        # [32,512]; one transpose chain produces hT_all [128,128] (f32r) used
        # as the next step's stationary operand.
        n_warm = int(os.environ.get("KERNEL_WARM", "6"))
        with tc.tile_pool(name="state", bufs=1) as st_p, \
             tc.tile_pool(name="xg", bufs=3) as xg_p, \
             tc.tile_pool(name="gt", bufs=3) as gt_p, \
             tc.tile_pool(name="gps", bufs=2, space="PSUM") as gps, \
             tc.tile_pool(name="tps", bufs=2, space="PSUM") as tps, \
             tc.tile_pool(name="dps", bufs=1, space="PSUM") as dps:

            c_sb = st_p.tile([BL, H], F32)
            nc.vector.memset(c_sb[:], 0.0)
            h_pad = st_p.tile([32, H], F32)
            nc.vector.memset(h_pad[:], 0.0)
            hT_all = st_p.tile([128, 128], F32R)
            nc.vector.memset(hT_all[:].bitcast(F32), 0.0)

            hs_v = hs_dram[:].rearrange("(b t) d -> b t d", t=T)
            pdum = dps.tile([32, 512], F32, name="pdum")

            # gate order: gg, i, f, o — the tanh chain starts as early as possible
            GATE_ORDER = (2, 0, 1, 3)
            for s in range(scan_steps if max_stage >= 3 else 1):
                xg_s = xg_p.tile([BL, G4], F32R, tag="xg")
                nc.sync.dma_start(out=xg_s[:], in_=xg_dram[s, :, :])

                acts = {}
                for gi, n in enumerate(GATE_ORDER):
                    pg = gps.tile([32, 512], F32, tag=f"pg{n}", bufs=1, name=f"pg{n}")
                    nc.tensor.matmul(
                        pg[:],
                        lhsT=ident_r[:BL, :32],
                        rhs=xg_s[:, 512 * n : 512 * (n + 1)],
                        start=True,
                        stop=False,
                    )
                    for k in range(4):
                        nc.tensor.matmul(
                            pg[:],
                            lhsT=hT_all[:, 32 * k : 32 * (k + 1)],
                            rhs=w_hhT_sb[k][:, 512 * n : 512 * (n + 1)],
                            start=False,
                            stop=(k == 3),
                        )
                    a_sb = gt_p.tile([BL, 512], F32, tag=f"a{n}")
                    if n == 1:  # f: halved so the c-chain pipelines
                        for hf in (0, 1):
                            sl = slice(256 * hf, 256 * (hf + 1))
                            nc.scalar.activation(
                                out=a_sb[:, sl], in_=pg[:BL, sl], func=AF.Sigmoid
                            )
                    else:
                        nc.scalar.activation(
                            out=a_sb[:],
                            in_=pg[:BL, :],
                            func=AF.Tanh if n == 2 else AF.Sigmoid,
                        )
                    acts[n] = a_sb
                    if ablate == "mmonly":
                        continue
                    if n == 0:  # after i (2nd group): t1 = sig_i * tanh_gg
                        t1 = gt_p.tile([BL, H], F32, tag="t1")
                        nc.vector.tensor_tensor(
                            out=t1[:], in0=acts[0][:], in1=acts[2][:], op=OP.mult
                        )
                    elif n == 1:  # after f (3rd group): c = c*f + t1; tanh(c)
                        tch = gt_p.tile([BL, H], F32, tag="tch")
                        for hf in (0, 1):
                            sl = slice(256 * hf, 256 * (hf + 1))
                            nc.vector.tensor_tensor(
                                out=c_sb[:, sl], in0=c_sb[:, sl], in1=acts[1][:, sl],
                                op=OP.mult,
                            )
                            nc.vector.tensor_tensor(
                                out=c_sb[:, sl], in0=c_sb[:, sl], in1=t1[:, sl],
                                op=OP.add,
                            )
                            nc.scalar.activation(
                                out=tch[:, sl], in_=c_sb[:, sl], func=AF.Tanh
                            )

                # PE warming filler while DVE/ACT run the tail
                for d in range(n_warm):
                    nc.tensor.matmul(
                        pdum[:],
                        lhsT=w_hhT_sb[0][:, :32],
                        rhs=w_hhT_sb[1][:, :512],
                        start=True,
                        stop=True,
                    )

                if ablate == "mmonly":
                    continue
                # h = sig_o * tanh(c)
                for hf in (0, 1):
                    sl = slice(256 * hf, 256 * (hf + 1))
                    nc.vector.tensor_tensor(
                        out=h_pad[:BL, sl], in0=acts[3][:, sl], in1=tch[:, sl],
                        op=OP.mult,
                    )
                nc.gpsimd.dma_start(out=hs_v[:, s, :], in_=h_pad[:BL, :])

                if ablate in ("notrans",):
                    continue
                pt_all = tps.tile([128, 128], F32, tag="pt")
                for q in range(4):
                    nc.tensor.transpose(
                        out=pt_all[:, 32 * q : 32 * (q + 1)],
                        in_=h_pad[:, 128 * q : 128 * (q + 1)],
                        identity=ident[:32, :32],
                    )
                if ablate == "nocopy":
                    continue
                nc.vector.tensor_copy(out=hT_all[:], in_=pt_all[:])

        whh_p.release()

        # ================= Stage 4: AllGather pair (fwd, bwd) ================
        if max_stage >= 4:
          nc.gpsimd.collective_compute(
            "AllGather",
            OP.bypass,
            replica_groups=[[0, 4], [1, 5], [2, 6], [3, 7]],
            ins=[hs_dram[:].opt()],
            outs=[hs_ag[:].opt()],
        )

        # ================= Stage 5: reorder-gather + transpose -> rnnT =======
        with tc.tile_pool(name="h1T", bufs=1) as h1_p, \
             tc.tile_pool(name="scT", bufs=1) as sc_p:

            h1T = [h1_p.tile([128, ROWS], F32R, tag=f"h1T{m}", name=f"h1T{m}") for m in range(4)]
            scoresT = sc_p.tile([C, SROWS], F32)

            with tc.tile_pool(name="rnnT", bufs=1) as rt_p:
                rnnT = [rt_p.tile([128, ROWS], F32R, tag=f"rnnT{j}", name=f"rnnT{j}") for j in range(8)]

                with tc.tile_pool(name="s5", bufs=3) as s5p, \
                     tc.tile_pool(name="s5ps", bufs=4, space="PSUM") as s5ps:
                    for cch in range(2 * ROWS // 128 if max_stage >= 5 else 0):
                        idx_sb = s5p.tile([128, 1], I32, tag="gidx")
                        nc.sync.dma_start(
                            out=idx_sb[:], in_=gidx_d[cch * 128 : (cch + 1) * 128, :]
                        )
                        t_sb = s5p.tile([128, H], F32, tag="hrows")
                        nc.gpsimd.indirect_dma_start(
                            out=t_sb[:],
                            out_offset=None,
                            in_=hs_ag[:, :],
                            in_offset=bass.IndirectOffsetOnAxis(ap=idx_sb[:, :1], axis=0),
                        )
                        g0 = cch * 128  # global gathered row
                        for k in range(4):
                            pt = s5ps.tile([128, 128], F32, tag="pt")
                            nc.tensor.transpose(
                                out=pt[:], in_=t_sb[:, 128 * k : 128 * (k + 1)],
                                identity=ident[:],
                            )
                            # rows g0..g0+128 map to (dir = g//ROWS, col = g%ROWS)
                            if (g0 // ROWS) == ((g0 + 127) // ROWS):
                                d = g0 // ROWS
                                if (cch + k) % 2 == 0:
                                    nc.vector.tensor_copy(
                                        out=rnnT[d * 4 + k][:, g0 % ROWS : g0 % ROWS + 128],
                                        in_=pt[:],
                                    )
                                else:
                                    nc.scalar.copy(
                                        out=rnnT[d * 4 + k][:, g0 % ROWS : g0 % ROWS + 128],
                                        in_=pt[:],
                                    )
                            else:  # chunk straddles the fwd/bwd boundary
                                n0 = ROWS - g0
                                nc.vector.tensor_copy(
                                    out=rnnT[k][:, g0 : g0 + n0], in_=pt[:, :n0]
                                )
                                nc.vector.tensor_copy(
                                    out=rnnT[4 + k][:, 0 : 128 - n0], in_=pt[:, n0:]
                                )

                # ================= Stage 6: h1T = relu(lin1) ====================
                with tc.tile_pool(name="s6w", bufs=1) as s6w, \
                     tc.tile_pool(name="s6ps", bufs=2, space="PSUM") as s6ps:

                    l1w = [s6w.tile([128, H], F32R, tag=f"l1w{k}", name=f"l1w{k}") for k in range(8)]
                    for k in range(8):
                        nc.sync.dma_start(
                            out=l1w[k][:], in_=lin1_wT[128 * k : 128 * (k + 1), :]
                        )
                    l1b = [s6w.tile([128, 1], F32, tag=f"l1b{m}", name=f"l1b{m}") for m in range(4)]
                    for m in range(4):
                        nc.sync.dma_start(
                            out=l1b[m][:], in_=lin1_b[128 * m : 128 * (m + 1), :]
                        )

                    for m in range(4 if max_stage >= 6 else 0):
                        for c0, cw in ((0, 512), (512, 448)):
                            ph = s6ps.tile([128, 512], F32, tag="ph")
                            for k in range(8):
                                nc.tensor.matmul(
                                    ph[:, :cw],
                                    lhsT=l1w[k][:, 128 * m : 128 * (m + 1)],
                                    rhs=rnnT[k][:, c0 : c0 + cw],
                                    start=(k == 0),
                                    stop=(k == 7),
                                )
                            nc.scalar.activation(
                                out=h1T[m][:, c0 : c0 + cw],
                                in_=ph[:, :cw],
                                func=AF.Relu,
                                bias=l1b[m][:],
                            )

            # ============ Stages 7-10: per-batch lin2+LN+span+label ==========
            with tc.tile_pool(name="s7w", bufs=1) as s7w, \
                 tc.tile_pool(name="s7", bufs=3) as s7p, \
                 tc.tile_pool(name="spT", bufs=1) as sp_p, \
                 tc.tile_pool(name="s7ps", bufs=2, space="PSUM") as s7ps:

                l2w = [s7w.tile([128, H], F32R, tag=f"l2w{k}", name=f"l2w{k}") for k in range(4)]
                for k in range(4):
                    nc.sync.dma_start(
                        out=l2w[k][:], in_=lin2_wT[128 * k : 128 * (k + 1), :]
                    )
                l2b = s7w.tile([T, H], F32)
                nc.gpsimd.dma_start(out=l2b[:], in_=lin2_b[:, :].to_broadcast([T, H]))
                lng = s7w.tile([T, H], F32)
                nc.gpsimd.dma_start(out=lng[:], in_=ln_g[:, :].to_broadcast([T, H]))
                lnb = s7w.tile([T, H], F32)
                nc.gpsimd.dma_start(out=lnb[:], in_=ln_b[:, :].to_broadcast([T, H]))
                lblw = [s7w.tile([128, C], F32R, tag=f"lblw{k}", name=f"lblw{k}") for k in range(4)]
                for k in range(4):
                    nc.sync.dma_start(
                        out=lblw[k][:], in_=label_w[128 * k : 128 * (k + 1), :]
                    )
                lblb = s7w.tile([C, 1], F32)
                nc.sync.dma_start(out=lblb[:], in_=label_b[:, :])
                SP = 1160  # S padded (fp32r matmul needs even free dims)
                mspan = s7w.tile([T, SP], F32R)
                nc.vector.memset(mspan[:].bitcast(F32), 0.0)
                nc.gpsimd.dma_start(out=mspan[:, :S], in_=mspanT_d[:, :])
                eps_sb = s7w.tile([T, 1], F32)
                nc.vector.memset(eps_sb[:], LN_EPS)

                SCH = ((0, 512), (512, 512), (1024, 136))
                for b in range(BL if max_stage >= 7 else 0):
                    ph2 = s7ps.tile([T, 512], F32, tag="ph2")
                    for k in range(4):
                        nc.tensor.matmul(
                            ph2[:],
                            lhsT=h1T[k][:, b * T : (b + 1) * T],
                            rhs=l2w[k][:],
                            start=(k == 0),
                            stop=(k == 3),
                        )
                    tr = s7p.tile([T, H], F32R, tag="tr")
                    nc.vector.tensor_tensor(
                        out=tr[:], in0=ph2[:],
                        in1=l2b[:], op=OP.add,
                    )
                    # LayerNorm over H
                    stats = s7p.tile([T, 6], F32, tag="stats")
                    nc.vector.bn_stats(out=stats[:], in_=tr[:])
                    mv = s7p.tile([T, 2], F32, tag="mv")
                    nc.vector.bn_aggr(out=mv[:], in_=stats[:])
                    sd = s7p.tile([T, 1], F32, tag="sd")
                    nc.scalar.activation(
                        out=sd[:], in_=mv[:, 1:2], func=AF.Sqrt, bias=eps_sb[:]
                    )
                    rstd = s7p.tile([T, 1], F32, tag="rstd")
                    nc.vector.reciprocal(out=rstd[:], in_=sd[:])
                    nc.vector.tensor_scalar(
                        out=tr[:], in0=tr[:],
                        scalar1=mv[:, 0:1], scalar2=rstd[:],
                        op0=OP.subtract, op1=OP.mult,
                    )
                    nc.vector.tensor_tensor(
                        out=tr[:], in0=tr[:],
                        in1=lng[:], op=OP.mult,
                    )
                    nc.vector.tensor_tensor(
                        out=tr[:], in0=tr[:],
                        in1=lnb[:], op=OP.add,
                    )
                    # span pooling (transposed): spansT[m] = tr[:,m-chunk].T @ mspanT
                    spansT = [
                        sp_p.tile([128, SP], F32R, tag=f"spansT{m}", name=f"spansT{m}")
                        for m in range(4)
                    ]
                    for m in range(4):
                        for ci, (n0, nw) in enumerate(SCH):
                            psp = s7ps.tile([128, 512], F32, tag="psp")
                            nc.tensor.matmul(
                                psp[:, :nw],
                                lhsT=tr[:, 128 * m : 128 * (m + 1)],
                                rhs=mspan[:, n0 : n0 + nw],
                                start=True,
                                stop=True,
                            )
                            if (m * 3 + ci) % 2 == 0:
                                nc.vector.tensor_copy(
                                    out=spansT[m][:, n0 : n0 + nw], in_=psp[:, :nw]
                                )
                            else:
                                nc.scalar.copy(
                                    out=spansT[m][:, n0 : n0 + nw], in_=psp[:, :nw]
                                )
                    # label scores (transposed): scoresT = label_w.T @ spansT
                    for n0, nw in SCH:
                        psc = s7ps.tile([C, 512], F32, tag="psc")
                        for k in range(4):
                            nc.tensor.matmul(
                                psc[:, :nw],
                                lhsT=lblw[k][:],
                                rhs=spansT[k][:, n0 : n0 + nw],
                                start=(k == 0),
                                stop=(k == 3),
                            )
                        w_real = min(nw, S - n0)
                        nc.scalar.activation(
                            out=scoresT[:, b * S + n0 : b * S + n0 + w_real],
                            in_=psc[:, :w_real],
                            func=AF.Identity,
                            bias=lblb[:],
                        )

                # ============ Stage 10: global log_softmax ===============
                cc_in = dram_p.tile([C, 32], F32)
                cc_out = dram_p.tile([C, 32], F32)
                cc_in2 = dram_p.tile([C, 32], F32)
                cc_out2 = dram_p.tile([C, 32], F32)
                RG = [[0, 1, 2, 3], [4, 5, 6, 7]]

                if max_stage < 10:
                    nc.vector.memset(scoresT[:, :4], 0.0)  # keep scoresT written
                lmax = s7p.tile([C, 1], F32, tag="lmax")
                nc.vector.tensor_reduce(
                    out=lmax[:], in_=scoresT[:], axis=mybir.AxisListType.X,
                    op=OP.max,
                )
                stg32 = s7p.tile([C, 32], F32, tag="stg32")
                nc.vector.tensor_copy(out=stg32[:], in_=lmax[:].to_broadcast([C, 32]))
                nc.sync.dma_start(out=cc_in[:, :], in_=stg32[:])
                gmax = s7p.tile([C, 32], F32, tag="gmax")
                if max_stage >= 10:
                    nc.gpsimd.collective_compute(
                        "AllReduce", OP.max, replica_groups=RG,
                        ins=[cc_in[:].opt()], outs=[cc_out[:].opt()],
                    )
                    nc.sync.dma_start(out=gmax[:], in_=cc_out[:, :])
                else:
                    nc.vector.tensor_copy(out=gmax[:], in_=stg32[:])

                negmax = s7p.tile([C, 1], F32, tag="negmax")
                nc.scalar.activation(
                    out=negmax[:], in_=gmax[:, 0:1], func=AF.Copy, scale=-1.0
                )
                partials = s7p.tile([C, BL], F32, tag="partials")
                for ch in range(BL):
                    expc = s7p.tile([C, S], F32, tag="expc")
                    nc.scalar.activation(
                        out=expc[:], in_=scoresT[:, ch * S : (ch + 1) * S],
                        func=AF.Exp, bias=negmax[:],
                        accum_out=partials[:, ch : ch + 1],
                    )
                lsum = s7p.tile([C, 1], F32, tag="lsum")
                nc.vector.tensor_reduce(
                    out=lsum[:], in_=partials[:], axis=mybir.AxisListType.X,
                    op=OP.add,
                )
                stg32b = s7p.tile([C, 32], F32, tag="stg32b")
                nc.vector.tensor_copy(
                    out=stg32b[:], in_=lsum[:].to_broadcast([C, 32])
                )
                nc.sync.dma_start(out=cc_in2[:, :], in_=stg32b[:])
                gsum = s7p.tile([C, 32], F32, tag="gsum")
                if max_stage >= 10:
                    nc.gpsimd.collective_compute(
                        "AllReduce", OP.add, replica_groups=RG,
                        ins=[cc_in2[:].opt()], outs=[cc_out2[:].opt()],
                    )
                    nc.sync.dma_start(out=gsum[:], in_=cc_out2[:, :])
                else:
                    nc.vector.tensor_copy(out=gsum[:], in_=stg32b[:])
                logz = s7p.tile([C, 1], F32, tag="logz")
                nc.scalar.activation(out=logz[:], in_=gsum[:, 0:1], func=AF.Ln)
                corr = s7p.tile([C, 1], F32, tag="corr")
                nc.vector.tensor_tensor(
                    out=corr[:], in0=gmax[:, 0:1], in1=logz[:], op=OP.add
                )
                normT = scoresT  # in place
                nc.vector.tensor_scalar(
                    out=normT[:], in0=scoresT[:], scalar1=corr[:], scalar2=None,
                    op0=OP.subtract,
                )

            # ============ Stage 11: transpose out ====================
            with tc.tile_pool(name="outp", bufs=1) as out_p, \
                 tc.tile_pool(name="ops", bufs=4, space="PSUM") as ops:
                n_full = SROWS // 128  # 72
                tail = SROWS - n_full * 128  # 24
                out_sb = out_p.tile([128, n_full * C], F32)
                out_tail = out_p.tile([tail, C], F32)
                for m in range(n_full + 1):
                    cw = 128 if m < n_full else tail
                    pto = ops.tile([128, C], F32, tag="pto")
                    nc.tensor.transpose(
                        out=pto[:cw, :],
                        in_=normT[:, 128 * m : 128 * m + cw],
                        identity=ident[:C, :C],
                    )
                    if m < n_full:
                        nc.vector.tensor_copy(
                            out=out_sb[:, m * C : (m + 1) * C], in_=pto[:, :]
                        )
                    else:
                        nc.vector.tensor_copy(
                            out=out_tail[:], in_=pto[:cw, :]
                        )
                out_v = out[: n_full * 128, :].rearrange(
                    "(m p) c -> p m c", p=128
                )
                nc.sync.dma_start(out=out_v, in_=out_sb[:].rearrange(
                    "p (m c) -> p m c", c=C))
                nc.sync.dma_start(
                    out=out[n_full * 128 :, :], in_=out_tail[:]
                )

        const_p.release()
        dram_p.release()

    nc.finalize()
    return nc


_PROGRAM = None


def _get_program():
    global _PROGRAM
    if _PROGRAM is None:
        _PROGRAM = build_program()
    return _PROGRAM


def _in_maps(inputs: dict) -> list[dict]:
    tokens = np.ascontiguousarray(np.asarray(inputs["tokens"], np.int32))
    emb = np.ascontiguousarray(np.asarray(inputs["emb"], np.float32))
    shared = {
        "emb": emb,
        "lin1_wT": np.ascontiguousarray(np.asarray(inputs["lin1_w"], np.float32).T),
        "lin1_b": np.ascontiguousarray(
            np.asarray(inputs["lin1_b"], np.float32)[:, None]
        ),
        "lin2_wT": np.ascontiguousarray(np.asarray(inputs["lin2_w"], np.float32).T),
        "lin2_b": np.ascontiguousarray(
            np.asarray(inputs["lin2_b"], np.float32)[None, :]
        ),
        "ln_g": np.ascontiguousarray(np.asarray(inputs["ln_g"], np.float32)[None, :]),
        "ln_b": np.ascontiguousarray(np.asarray(inputs["ln_b"], np.float32)[None, :]),
        "label_w": np.ascontiguousarray(np.asarray(inputs["label_w"], np.float32)),
        "label_b": np.ascontiguousarray(
            np.asarray(inputs["label_b"], np.float32)[:, None]
        ),
    }
    per_dir = {}
    for d, sfx in ((0, "f"), (1, "b")):
        per_dir[d] = {
            "w_ihT": np.ascontiguousarray(
                np.asarray(inputs[f"w_ih_{sfx}"], np.float32).T
            ),
            "w_hhT": np.ascontiguousarray(
                np.asarray(inputs[f"w_hh_{sfx}"], np.float32).T
            ),
            "gbias": np.ascontiguousarray(
                (
                    np.asarray(inputs[f"b_ih_{sfx}"], np.float32)
                    + np.asarray(inputs[f"b_hh_{sfx}"], np.float32)
                )[None, :]
            ),
        }
    maps = []
    for core in range(NCORES):
        d = core // 4
        g = core % 4
        tk = tokens[g * BL : (g + 1) * BL]  # [8, 120]
        if d == 1:
            tk = tk[:, ::-1]
        m = dict(shared)
        m.update(per_dir[d])
        m["tok"] = np.ascontiguousarray(tk.reshape(-1)[:, None])
        maps.append(m)
    return maps


def kernel(**inputs) -> np.ndarray:
    nc = _get_program()
    res = run_bass_kernel_spmd(
        nc,
        _in_maps(inputs),
        core_ids=list(range(NCORES)),
        trace=bool(int(os.environ.get("KERNEL_TRACE", "0"))),
    )
    kernel.last_results = res
    outs = [res.results[g]["out"] for g in range(4)]
    return np.ascontiguousarray(np.concatenate(outs, axis=0))



# revision 13
# speedup vs baseline: 1.1175x; 1.1175x over previous
"""Trainium2 Bass kernel for nn_Luban7_29609504539316 (BiLSTM + span pool + log_softmax).

Sharding (8 cores):
  - Direction-split scan: cores 0-3 run the FORWARD LSTM, cores 4-7 the BACKWARD
    LSTM (fed host-reversed tokens).  Core c handles batch group g = c % 4
    (batches g*8 .. g*8+8) for the scan.
  - Pair (c, c+4) exchanges hidden states (bf16, two time-chunked AllGathers
    issued mid-scan so the collective overlaps the remaining scan steps).
  - Post-LSTM stages are BATCH-SPLIT across the pair: core c handles the first
    4 batches of its group, core c+4 the last 4 (per-core gather-index input).
  - log_softmax over the span axis is single-pass (scores are bounded ~|4.3|):
    local exp-sums are AllReduce-summed over all 8 cores; the log-Z subtraction
    is folded into the output transpose copies.
  - Host concatenates the outputs of all 8 cores in batch order.

The program is identical on all cores (SPMD); direction and batch assignment
live entirely in the per-core input data (tokens, per-direction weights,
gather indices).
"""

import os
import sys

import numpy as np

for _p in ("/opt/trn_rl_repo",):
    if _p not in sys.path and os.path.isdir(_p):
        sys.path.insert(0, _p)

import concourse.bass as bass
import concourse.tile as tile
from concourse import bacc
from concourse import mybir
from concourse.bass_utils import run_bass_kernel_spmd

F32 = mybir.dt.float32
F32R = mybir.dt.float32r
BF16 = mybir.dt.bfloat16
FP8 = mybir.dt.float8e4
I32 = mybir.dt.int32
AF = mybir.ActivationFunctionType
OP = mybir.AluOpType
PM = mybir.MatmulPerfMode
USE_FP8 = bool(int(os.environ.get("KERNEL_FP8", "0")))

# Problem dims (hardcoded per spec)
B, T, V, E, H, C, L = 32, 120, 32000, 256, 512, 20, 10
G4 = 4 * H  # 2048
LN_EPS = 1e-5
NCORES = 8
BL = 8             # batches per scan core
PB = 4             # post-stage batches per core
ROWS = BL * T      # 960
PROWS = PB * T     # 480  (post-stage rows per dir)
CH = 60            # scan steps per AllGather chunk
NCH = T // CH      # 2

# static span table (matches reference loop order)
_begs, _lens = [], []
for _b in range(T):
    for _l in range(1, min(L, T - _b) + 1):
        _begs.append(_b)
        _lens.append(_l)
BEGS = np.asarray(_begs, np.int32)
LENS = np.asarray(_lens, np.int32)
S = len(_begs)  # 1155
assert S == 1155
SROWS = PB * S     # output rows per core = 4620


def _mspanT() -> np.ndarray:
    """[T, S] span-mean pooling matrix (inv_len folded in)."""
    m = np.zeros((T, S), np.float32)
    for s in range(S):
        m[BEGS[s] : BEGS[s] + LENS[s], s] = 1.0 / LENS[s]
    return m


def _gather_idx(is_bwd_core: bool) -> np.ndarray:
    """[2*PROWS, 1] per-core gather table into hs_ag [2*ROWS rows].

    hs_ag row layout (identical on both cores of a pair):
      row = 960*j + 480*r + (s - 60*j)*8 + b_local
    where j = chunk (s//60), r = rank in pair (0=fwd core, 1=bwd core),
    s = scan step, b_local = batch index within the group (0..7).

    Gather order (defines rnnT columns): first fwd rows (pb-major, t-minor),
    then bwd rows.  For the bwd direction scan step s corresponds to time
    t = T-1-s, so we read row for s = T-1-t.
    """
    idx = np.empty(2 * PROWS, np.int32)
    boff = PB if is_bwd_core else 0
    for pb in range(PB):
        bl = boff + pb
        for t in range(T):
            s = t
            j = s // CH
            idx[pb * T + t] = ROWS * j + (s - CH * j) * BL + bl
            s = T - 1 - t
            j = s // CH
            idx[PROWS + pb * T + t] = ROWS * j + PROWS + (s - CH * j) * BL + bl
    return idx[:, None]


def _r(ap):
    return ap.bitcast(F32R)


def build_program():
    max_stage = int(os.environ.get("KERNEL_MAX_STAGE", "99"))
    scan_steps = int(os.environ.get("KERNEL_SCAN_STEPS", str(T)))
    n_warm = int(os.environ.get("KERNEL_WARM", "0"))
    nc = bacc.Bacc(trn_type="TRN2", num_devices=NCORES)

    # ---- I/O ----
    tok = nc.dram_tensor("tok", [ROWS, 1], I32, kind="ExternalInput")
    gidx = nc.dram_tensor("gidx", [2 * PROWS, 1], I32, kind="ExternalInput")
    emb = nc.dram_tensor("emb", [V, E], F32, kind="ExternalInput")
    w_ihT = nc.dram_tensor("w_ihT", [E, G4], F32R, kind="ExternalInput")
    w_hhT = nc.dram_tensor("w_hhT", [H, G4], F32R, kind="ExternalInput")
    gbias = nc.dram_tensor("gbias", [1, G4], F32, kind="ExternalInput")
    lin1_wT = nc.dram_tensor("lin1_wT", [2 * H, H], BF16, kind="ExternalInput")
    lin1_b = nc.dram_tensor("lin1_b", [H, 1], F32, kind="ExternalInput")
    lin2_wT = nc.dram_tensor("lin2_wT", [H, H], F32R, kind="ExternalInput")
    lin2_b = nc.dram_tensor("lin2_b", [1, H], F32, kind="ExternalInput")
    ln_g = nc.dram_tensor("ln_g", [1, H], F32, kind="ExternalInput")
    ln_b = nc.dram_tensor("ln_b", [1, H], F32, kind="ExternalInput")
    label_w = nc.dram_tensor("label_w", [H, C], F32R, kind="ExternalInput")
    label_b = nc.dram_tensor("label_b", [C, 1], F32, kind="ExternalInput")
    out = nc.dram_tensor("out", [SROWS, C], F32, kind="ExternalOutput")

    # ---- inline constants (same on every core) ----
    ident_d = nc.inline_tensor(np.eye(128, dtype=np.float32), name="ident")
    mspanT_d = nc.inline_tensor(_mspanT(), name="mspanT")

    with tile.TileContext(nc) as tc:
        # long-lived pools (released at end of build)
        const_p = tc.alloc_tile_pool(name="const", bufs=1)
        whh_p = tc.alloc_tile_pool(name="whh", bufs=1)
        dram_p = tc.alloc_tile_pool(name="dram", bufs=1, space="DRAM")

        ident = const_p.tile([128, 128], F32)
        nc.sync.dma_start(out=ident[:], in_=ident_d[:, :])
        ident_r = const_p.tile([128, 128], F32R)
        nc.gpsimd.dma_start(out=ident_r[:], in_=ident_d[:, :])
        ident_bf = const_p.tile([128, 128], BF16)
        nc.gpsimd.dma_start(out=ident_bf[:], in_=ident_d[:, :])

        w_hhT_sb = [whh_p.tile([128, G4], F32R, tag=f"whh{k}", name=f"whh{k}") for k in range(4)]
        for k in range(4):
            nc.sync.dma_start(out=w_hhT_sb[k][:], in_=w_hhT[128 * k : 128 * (k + 1), :])

        xg_dram = dram_p.tile([T, BL, G4], F32R)
        # per-chunk hs (bf16, t-major) + AllGather output
        hs_dram = [dram_p.tile([CH * BL, H], BF16, name=f"hsd{j}") for j in range(NCH)]
        hs_ag = dram_p.tile([2 * ROWS, H], BF16)

        # ================= Stage 1: embedding gather + transpose =============
        with tc.tile_pool(name="s1", bufs=3) as s1p, \
             tc.tile_pool(name="s1ps", bufs=4, space="PSUM") as s1ps, \
             tc.tile_pool(name="xt", bufs=1) as xt_p, \
             tc.tile_pool(name="wih", bufs=1) as wih_p:

            xT = [xt_p.tile([128, ROWS], F32R, tag=f"xT{k}", name=f"xT{k}") for k in range(2)]
            w_ihT_sb = [wih_p.tile([128, G4], F32R, tag=f"wih{k}", name=f"wih{k}") for k in range(2)]
            gbias_sb = wih_p.tile([T, G4], F32)
            for k in range(2):
                nc.sync.dma_start(
                    out=w_ihT_sb[k][:], in_=w_ihT[128 * k : 128 * (k + 1), :]
                )
            nc.gpsimd.dma_start(
                out=gbias_sb[:], in_=gbias[:, :].to_broadcast([T, G4])
            )

            n_full = ROWS // 128          # 7
            tail = ROWS - n_full * 128    # 64
            for r in range(n_full + 1):
                rows = 128 if r < n_full else tail
                idx_sb = s1p.tile([128, 1], I32, tag="idx")
                x_sb = s1p.tile([128, E], F32, tag="x")
                nc.sync.dma_start(
                    out=idx_sb[:rows], in_=tok[r * 128 : r * 128 + rows, :]
                )
                nc.gpsimd.indirect_dma_start(
                    out=x_sb[:rows, :],
                    out_offset=None,
                    in_=emb[:, :],
                    in_offset=bass.IndirectOffsetOnAxis(ap=idx_sb[:rows, :1], axis=0),
                )
                for k in range(2):
                    pt = s1ps.tile([128, 128], F32, tag="pt")
                    nc.tensor.transpose(
                        out=pt[:, :rows],
                        in_=x_sb[:rows, 128 * k : 128 * (k + 1)],
                        identity=ident[:rows, :rows],
                    )
                    nc.vector.tensor_copy(
                        out=xT[k][:, r * 128 : r * 128 + rows], in_=pt[:, :rows]
                    )

            # ================= Stage 2: xg = x @ w_ihT + bias ================
            with tc.tile_pool(name="s2", bufs=3) as s2p, \
                 tc.tile_pool(name="s2ps", bufs=3, space="PSUM") as s2ps:
                for b in range(BL):
                    for n in range(4):
                        ps = s2ps.tile([T, 512], F32, tag="ps")
                        for k in range(2):
                            nc.tensor.matmul(
                                ps[:],
                                lhsT=xT[k][:, b * T : (b + 1) * T],
                                rhs=w_ihT_sb[k][:, 512 * n : 512 * (n + 1)],
                                start=(k == 0),
                                stop=(k == 1),
                            )
                        stg = s2p.tile([T, 512], F32R, tag="stg")
                        nc.vector.tensor_tensor(
                            out=stg[:],
                            in0=ps[:],
                            in1=gbias_sb[:, 512 * n : 512 * (n + 1)],
                            op=OP.add,
                        )
                        nc.sync.dma_start(
                            out=xg_dram[:, b, 512 * n : 512 * (n + 1)], in_=stg[:]
                        )

        # ================= Stage 3: LSTM scan (this core's direction) ========
        # Layout: one PSUM tile per gate pg[32,512]; batch padded 8->32 with
        # zeros so every read row is defined.  The xg contribution is injected
        # by an identity-matmul per gate.  h lives in h_pad [32,512]; one
        # transpose chain produces hT_all [128,128] (f32r) used as the next
        # step's stationary operand.  hs is stored bf16 (cast during DMA) into
        # per-chunk DRAM tiles; each chunk AllGathers with the pair core as
        # soon as its last step is stored, overlapping the remaining scan.
        with tc.tile_pool(name="state", bufs=1) as st_p, \
             tc.tile_pool(name="xg", bufs=3) as xg_p, \
             tc.tile_pool(name="gt", bufs=3) as gt_p, \
             tc.tile_pool(name="gps", bufs=2, space="PSUM") as gps, \
             tc.tile_pool(name="tps", bufs=2, space="PSUM") as tps, \
             tc.tile_pool(name="dps", bufs=1, space="PSUM") as dps:

            c_sb = st_p.tile([BL, H], F32)
            nc.vector.memset(c_sb[:], 0.0)
            # h_pad rotates through several buffers so the hs-store DMA (and
            # the AllGather blocking the gpsimd queue mid-scan) never stalls
            # the next step's h write.
            NHP = 6
            h_pads = [st_p.tile([32, H], F32, tag=f"hp{i}", name=f"hp{i}") for i in range(NHP)]
            for i in range(NHP):
                nc.vector.memset(h_pads[i][:], 0.0)
            hT_all = st_p.tile([128, 128], F32R)
            nc.vector.memset(hT_all[:].bitcast(F32), 0.0)

            pdum = dps.tile([32, 512], F32, name="pdum")

            # gate order: gg, i, f, o — the tanh chain starts as early as possible
            GATE_ORDER = (2, 0, 1, 3)
            for s in range(scan_steps if max_stage >= 3 else 1):
                h_pad = h_pads[s % NHP]
                xg_s = xg_p.tile([BL, G4], F32R, tag="xg")
                nc.sync.dma_start(out=xg_s[:], in_=xg_dram[s, :, :])

                acts = {}
                for gi, n in enumerate(GATE_ORDER):
                    pg = gps.tile([32, 512], F32, tag=f"pg{n}", bufs=1, name=f"pg{n}")
                    nc.tensor.matmul(
                        pg[:],
                        lhsT=ident_r[:BL, :32],
                        rhs=xg_s[:, 512 * n : 512 * (n + 1)],
                        start=True,
                        stop=False,
                    )
                    for k in range(4):
                        nc.tensor.matmul(
                            pg[:],
                            lhsT=hT_all[:, 32 * k : 32 * (k + 1)],
                            rhs=w_hhT_sb[k][:, 512 * n : 512 * (n + 1)],
                            start=False,
                            stop=(k == 3),
                        )
                    a_sb = gt_p.tile([BL, 512], F32, tag=f"a{n}")
                    if n == 1:  # f: halved so the c-chain pipelines
                        for hf in (0, 1):
                            sl = slice(256 * hf, 256 * (hf + 1))
                            nc.scalar.activation(
                                out=a_sb[:, sl], in_=pg[:BL, sl], func=AF.Sigmoid
                            )
                    else:
                        nc.scalar.activation(
                            out=a_sb[:],
                            in_=pg[:BL, :],
                            func=AF.Tanh if n == 2 else AF.Sigmoid,
                        )
                    acts[n] = a_sb
                    if n == 0:  # after i (2nd group): t1 = sig_i * tanh_gg
                        t1 = gt_p.tile([BL, H], F32, tag="t1")
                        nc.vector.tensor_tensor(
                            out=t1[:], in0=acts[0][:], in1=acts[2][:], op=OP.mult
                        )
                    elif n == 1:  # after f (3rd group): c = c*f + t1; tanh(c)
                        tch = gt_p.tile([BL, H], F32, tag="tch")
                        for hf in (0, 1):
                            sl = slice(256 * hf, 256 * (hf + 1))
                            nc.vector.tensor_tensor(
                                out=c_sb[:, sl], in0=c_sb[:, sl], in1=acts[1][:, sl],
                                op=OP.mult,
                            )
                            nc.vector.tensor_tensor(
                                out=c_sb[:, sl], in0=c_sb[:, sl], in1=t1[:, sl],
                                op=OP.add,
                            )
                            nc.scalar.activation(
                                out=tch[:, sl], in_=c_sb[:, sl], func=AF.Tanh
                            )

                # optional PE warming filler
                for _ in range(n_warm):
                    nc.tensor.matmul(
                        pdum[:],
                        lhsT=w_hhT_sb[0][:, :32],
                        rhs=w_hhT_sb[1][:, :512],
                        start=True,
                        stop=True,
                    )

                # h = sig_o * tanh(c)
                for hf in (0, 1):
                    sl = slice(256 * hf, 256 * (hf + 1))
                    nc.vector.tensor_tensor(
                        out=h_pad[:BL, sl], in0=acts[3][:, sl], in1=tch[:, sl],
                        op=OP.mult,
                    )
                nc.gpsimd.dma_start(
                    out=hs_dram[s // CH][(s % CH) * BL : (s % CH + 1) * BL, :],
                    in_=h_pad[:BL, :],
                )

                pt_all = tps.tile([128, 128], F32, tag="pt")
                for q in range(4):
                    nc.tensor.transpose(
                        out=pt_all[:, 32 * q : 32 * (q + 1)],
                        in_=h_pad[:, 128 * q : 128 * (q + 1)],
                        identity=ident[:32, :32],
                    )
                nc.vector.tensor_copy(out=hT_all[:], in_=pt_all[:])

                # ====== Stage 4: chunked AllGather with the pair core =======
                if max_stage >= 4 and (s + 1) % CH == 0 and s + 1 <= NCH * CH:
                    j = (s + 1) // CH - 1
                    nc.gpsimd.collective_compute(
                        "AllGather",
                        OP.bypass,
                        replica_groups=[[0, 4], [1, 5], [2, 6], [3, 7]],
                        ins=[hs_dram[j][:].opt()],
                        outs=[hs_ag[ROWS * j : ROWS * (j + 1), :].opt()],
                    )

        whh_p.release()

        # ================= Stage 5: reorder-gather + transpose -> rnnT =======
        with tc.tile_pool(name="h1T", bufs=1) as h1_p, \
             tc.tile_pool(name="scT", bufs=1) as sc_p:

            h1T = [h1_p.tile([128, PROWS], F32R, tag=f"h1T{m}", name=f"h1T{m}") for m in range(4)]
            scoresT = sc_p.tile([C, SROWS], F32)

            with tc.tile_pool(name="rnnT", bufs=1) as rt_p:
                rnnT = [rt_p.tile([128, PROWS], BF16, tag=f"rnnT{j}", name=f"rnnT{j}") for j in range(8)]

                with tc.tile_pool(name="s5", bufs=3) as s5p, \
                     tc.tile_pool(name="s5ps", bufs=4, space="PSUM") as s5ps:
                    n_ch = (2 * PROWS + 127) // 128  # 8 (last chunk 64 rows)
                    for cch in range(n_ch if max_stage >= 5 else 0):
                        rows = min(128, 2 * PROWS - cch * 128)
                        idx_sb = s5p.tile([128, 1], I32, tag="gidx")
                        nc.sync.dma_start(
                            out=idx_sb[:rows],
                            in_=gidx[cch * 128 : cch * 128 + rows, :],
                        )
                        t_sb = s5p.tile([128, H], BF16, tag="hrows")
                        nc.gpsimd.indirect_dma_start(
                            out=t_sb[:rows, :],
                            out_offset=None,
                            in_=hs_ag[:, :],
                            in_offset=bass.IndirectOffsetOnAxis(ap=idx_sb[:rows, :1], axis=0),
                        )
                        g0 = cch * 128  # global gathered row
                        for k in range(4):
                            pt = s5ps.tile([128, 128], BF16, tag="pt")
                            nc.tensor.transpose(
                                out=pt[:, :rows], in_=t_sb[:rows, 128 * k : 128 * (k + 1)],
                                identity=ident_bf[:rows, :rows],
                            )
                            # rows g0..g0+rows map to (dir = g//PROWS, col = g%PROWS)
                            if (g0 // PROWS) == ((g0 + rows - 1) // PROWS):
                                dd = g0 // PROWS
                                nc.vector.tensor_copy(
                                    out=rnnT[dd * 4 + k][:, g0 % PROWS : g0 % PROWS + rows],
                                    in_=pt[:, :rows],
                                )
                            else:  # chunk straddles the fwd/bwd boundary
                                n0 = PROWS - g0
                                nc.vector.tensor_copy(
                                    out=rnnT[k][:, g0 : g0 + n0], in_=pt[:, :n0]
                                )
                                nc.vector.tensor_copy(
                                    out=rnnT[4 + k][:, 0 : rows - n0], in_=pt[:, n0:rows]
                                )

                # ================= Stage 6: h1T = relu(lin1) ====================
                with tc.tile_pool(name="s6w", bufs=1) as s6w, \
                     tc.tile_pool(name="s6ps", bufs=2, space="PSUM") as s6ps:

                    l1w = [s6w.tile([128, H], BF16, tag=f"l1w{k}", name=f"l1w{k}") for k in range(8)]
                    for k in range(8):
                        nc.sync.dma_start(
                            out=l1w[k][:], in_=lin1_wT[128 * k : 128 * (k + 1), :]
                        )
                    l1b = [s6w.tile([128, 1], F32, tag=f"l1b{m}", name=f"l1b{m}") for m in range(4)]
                    for m in range(4):
                        nc.sync.dma_start(
                            out=l1b[m][:], in_=lin1_b[128 * m : 128 * (m + 1), :]
                        )

                    for m in range(4 if max_stage >= 6 else 0):
                        ph = s6ps.tile([128, PROWS], F32, tag="ph")
                        for k in range(8):
                            nc.tensor.matmul(
                                ph[:],
                                lhsT=l1w[k][:, 128 * m : 128 * (m + 1)],
                                rhs=rnnT[k][:],
                                start=(k == 0),
                                stop=(k == 7),
                            )
                        nc.scalar.activation(
                            out=h1T[m][:],
                            in_=ph[:],
                            func=AF.Relu,
                            bias=l1b[m][:],
                        )

            # ============ Stages 7-10: per-batch lin2+LN+span+label ==========
            with tc.tile_pool(name="s7w", bufs=1) as s7w, \
                 tc.tile_pool(name="s7", bufs=3) as s7p, \
                 tc.tile_pool(name="spT", bufs=1) as sp_p, \
                 tc.tile_pool(name="s7ps", bufs=2, space="PSUM") as s7ps:

                l2w = [s7w.tile([128, H], F32R, tag=f"l2w{k}", name=f"l2w{k}") for k in range(4)]
                for k in range(4):
                    nc.sync.dma_start(
                        out=l2w[k][:], in_=lin2_wT[128 * k : 128 * (k + 1), :]
                    )
                l2b = s7w.tile([T, H], F32)
                nc.gpsimd.dma_start(out=l2b[:], in_=lin2_b[:, :].to_broadcast([T, H]))
                lng = s7w.tile([T, H], F32)
                nc.gpsimd.dma_start(out=lng[:], in_=ln_g[:, :].to_broadcast([T, H]))
                lnb = s7w.tile([T, H], F32)
                nc.gpsimd.dma_start(out=lnb[:], in_=ln_b[:, :].to_broadcast([T, H]))
                lblw = [s7w.tile([128, C], F32R, tag=f"lblw{k}", name=f"lblw{k}") for k in range(4)]
                for k in range(4):
                    nc.sync.dma_start(
                        out=lblw[k][:], in_=label_w[128 * k : 128 * (k + 1), :]
                    )
                lblb = s7w.tile([C, 1], F32)
                nc.sync.dma_start(out=lblb[:], in_=label_b[:, :])
                SP = 1160  # S padded (fp32r matmul needs even free dims)
                mspan = s7w.tile([T, SP], F32R)
                nc.vector.memset(mspan[:].bitcast(F32), 0.0)
                nc.gpsimd.dma_start(out=mspan[:, :S], in_=mspanT_d[:, :])
                eps_sb = s7w.tile([T, 1], F32)
                nc.vector.memset(eps_sb[:], LN_EPS)
                partials = s7w.tile([C, PB], F32)

                SCH = ((0, 512), (512, 512), (1024, 136))
                for b in range(PB if max_stage >= 7 else 0):
                    ph2 = s7ps.tile([T, 512], F32, tag="ph2")
                    for k in range(4):
                        nc.tensor.matmul(
                            ph2[:],
                            lhsT=h1T[k][:, b * T : (b + 1) * T],
                            rhs=l2w[k][:],
                            start=(k == 0),
                            stop=(k == 3),
                        )
                    tr = s7p.tile([T, H], F32R, tag="tr")
                    nc.vector.tensor_tensor(
                        out=tr[:], in0=ph2[:],
                        in1=l2b[:], op=OP.add,
                    )
                    # LayerNorm over H
                    stats = s7p.tile([T, 6], F32, tag="stats")
                    nc.vector.bn_stats(out=stats[:], in_=tr[:])
                    mv = s7p.tile([T, 2], F32, tag="mv")
                    nc.vector.bn_aggr(out=mv[:], in_=stats[:])
                    sd = s7p.tile([T, 1], F32, tag="sd")
                    nc.scalar.activation(
                        out=sd[:], in_=mv[:, 1:2], func=AF.Sqrt, bias=eps_sb[:]
                    )
                    rstd = s7p.tile([T, 1], F32, tag="rstd")
                    nc.vector.reciprocal(out=rstd[:], in_=sd[:])
                    nc.vector.tensor_scalar(
                        out=tr[:], in0=tr[:],
                        scalar1=mv[:, 0:1], scalar2=rstd[:],
                        op0=OP.subtract, op1=OP.mult,
                    )
                    nc.vector.tensor_tensor(
                        out=tr[:], in0=tr[:],
                        in1=lng[:], op=OP.mult,
                    )
                    nc.vector.tensor_tensor(
                        out=tr[:], in0=tr[:],
                        in1=lnb[:], op=OP.add,
                    )
                    # span pooling (transposed): spansT[m] = tr[:,m-chunk].T @ mspanT
                    spansT = [
                        sp_p.tile([128, SP], F32R, tag=f"spansT{m}", name=f"spansT{m}")
                        for m in range(4)
                    ]
                    for m in range(4):
                        for ci, (n0, nw) in enumerate(SCH):
                            psp = s7ps.tile([128, 512], F32, tag="psp")
                            nc.tensor.matmul(
                                psp[:, :nw],
                                lhsT=tr[:, 128 * m : 128 * (m + 1)],
                                rhs=mspan[:, n0 : n0 + nw],
                                start=True,
                                stop=True,
                            )
                            if (m * 3 + ci) % 2 == 0:
                                nc.vector.tensor_copy(
                                    out=spansT[m][:, n0 : n0 + nw], in_=psp[:, :nw]
                                )
                            else:
                                nc.scalar.copy(
                                    out=spansT[m][:, n0 : n0 + nw], in_=psp[:, :nw]
                                )
                    # label scores (transposed): scoresT = label_w.T @ spansT
                    for n0, nw in SCH:
                        psc = s7ps.tile([C, 512], F32, tag="psc")
                        for k in range(4):
                            nc.tensor.matmul(
                                psc[:, :nw],
                                lhsT=lblw[k][:],
                                rhs=spansT[k][:, n0 : n0 + nw],
                                start=(k == 0),
                                stop=(k == 3),
                            )
                        w_real = min(nw, S - n0)
                        nc.scalar.activation(
                            out=scoresT[:, b * S + n0 : b * S + n0 + w_real],
                            in_=psc[:, :w_real],
                            func=AF.Identity,
                            bias=lblb[:],
                        )
                    # single-pass softmax: exp-sum of this batch's scores
                    expc = s7p.tile([C, S], F32, tag="expc")
                    nc.scalar.activation(
                        out=expc[:], in_=scoresT[:, b * S : (b + 1) * S],
                        func=AF.Exp,
                        accum_out=partials[:, b : b + 1],
                    )

                # ============ Stage 10: global log-sum-exp ===============
                cc_in = dram_p.tile([C, 32], F32)
                cc_out = dram_p.tile([C, 32], F32, addr_space="Shared")
                RG = [[0, 1, 2, 3, 4, 5, 6, 7]]

                if max_stage < 10:
                    nc.vector.memset(scoresT[:, :4], 0.0)  # keep scoresT written
                lsum = s7p.tile([C, 1], F32, tag="lsum")
                nc.vector.tensor_reduce(
                    out=lsum[:], in_=partials[:], axis=mybir.AxisListType.X,
                    op=OP.add,
                )
                stg32 = s7p.tile([C, 32], F32, tag="stg32")
                nc.vector.tensor_copy(out=stg32[:], in_=lsum[:].to_broadcast([C, 32]))
                nc.sync.dma_start(out=cc_in[:, :], in_=stg32[:])
                gsum = s7p.tile([C, 32], F32, tag="gsum")
                if max_stage >= 10:
                    nc.gpsimd.collective_compute(
                        "AllReduce", OP.add, replica_groups=RG,
                        ins=[cc_in[:].opt()], outs=[cc_out[:].opt()],
                    )
                    nc.sync.dma_start(out=gsum[:], in_=cc_out[:, :])
                else:
                    nc.vector.tensor_copy(out=gsum[:], in_=stg32[:])
                logz = s7p.tile([C, 1], F32, tag="logz")
                nc.scalar.activation(out=logz[:], in_=gsum[:, 0:1], func=AF.Ln)
                normT = scoresT  # in place
                nc.vector.tensor_scalar(
                    out=normT[:], in0=scoresT[:], scalar1=logz[:], scalar2=None,
                    op0=OP.subtract,
                )

            # ============ Stage 11: transpose out ====================
            with tc.tile_pool(name="outp", bufs=1) as out_p, \
                 tc.tile_pool(name="ops", bufs=4, space="PSUM") as ops:
                n_full = SROWS // 128  # 36
                tail = SROWS - n_full * 128  # 12
                out_sb = out_p.tile([128, n_full * C], F32)
                out_tail = out_p.tile([tail, C], F32)
                for m in range(n_full + 1):
                    cw = 128 if m < n_full else tail
                    pto = ops.tile([128, C], F32, tag="pto")
                    nc.tensor.transpose(
                        out=pto[:cw, :],
                        in_=normT[:, 128 * m : 128 * m + cw],
                        identity=ident[:C, :C],
                    )
                    if m < n_full:
                        if m % 2 == 0:
                            nc.vector.tensor_copy(
                                out=out_sb[:, m * C : (m + 1) * C], in_=pto[:, :]
                            )
                        else:
                            nc.scalar.copy(
                                out=out_sb[:, m * C : (m + 1) * C], in_=pto[:, :]
                            )
                    else:
                        nc.vector.tensor_copy(
                            out=out_tail[:], in_=pto[:cw, :]
                        )
                out_v = out[: n_full * 128, :].rearrange(
                    "(m p) c -> p m c", p=128
                )
                nc.sync.dma_start(out=out_v, in_=out_sb[:].rearrange(
                    "p (m c) -> p m c", c=C))
                nc.sync.dma_start(
                    out=out[n_full * 128 :, :], in_=out_tail[:]
                )

        const_p.release()
        dram_p.release()

    nc.finalize()
    return nc


_PROGRAM = None


def _get_program():
    global _PROGRAM
    if _PROGRAM is None:
        _PROGRAM = build_program()
    return _PROGRAM


def _in_maps(inputs: dict) -> list[dict]:
    import ml_dtypes

    tokens = np.ascontiguousarray(np.asarray(inputs["tokens"], np.int32))
    emb = np.ascontiguousarray(np.asarray(inputs["emb"], np.float32))
    shared = {
        "emb": emb,
        "lin1_wT": np.ascontiguousarray(
            np.asarray(inputs["lin1_w"], np.float32).T.astype(ml_dtypes.bfloat16)
        ),
        "lin1_b": np.ascontiguousarray(
            np.asarray(inputs["lin1_b"], np.float32)[:, None]
        ),
        "lin2_wT": np.ascontiguousarray(np.asarray(inputs["lin2_w"], np.float32).T),
        "lin2_b": np.ascontiguousarray(
            np.asarray(inputs["lin2_b"], np.float32)[None, :]
        ),
        "ln_g": np.ascontiguousarray(np.asarray(inputs["ln_g"], np.float32)[None, :]),
        "ln_b": np.ascontiguousarray(np.asarray(inputs["ln_b"], np.float32)[None, :]),
        "label_w": np.ascontiguousarray(np.asarray(inputs["label_w"], np.float32)),
        "label_b": np.ascontiguousarray(
            np.asarray(inputs["label_b"], np.float32)[:, None]
        ),
    }
    per_dir = {}
    for d, sfx in ((0, "f"), (1, "b")):
        per_dir[d] = {
            "w_ihT": np.ascontiguousarray(
                np.asarray(inputs[f"w_ih_{sfx}"], np.float32).T
            ),
            "w_hhT": np.ascontiguousarray(
                np.asarray(inputs[f"w_hh_{sfx}"], np.float32).T
            ),
            "gbias": np.ascontiguousarray(
                (
                    np.asarray(inputs[f"b_ih_{sfx}"], np.float32)
                    + np.asarray(inputs[f"b_hh_{sfx}"], np.float32)
                )[None, :]
            ),
        }
    gidx_f = _gather_idx(False)
    gidx_b = _gather_idx(True)
    maps = []
    for core in range(NCORES):
        d = core // 4
        g = core % 4
        tk = tokens[g * BL : (g + 1) * BL]  # [8, 120]
        if d == 1:
            tk = tk[:, ::-1]
        m = dict(shared)
        m.update(per_dir[d])
        m["tok"] = np.ascontiguousarray(tk.reshape(-1)[:, None])
        m["gidx"] = np.ascontiguousarray(gidx_b if d == 1 else gidx_f)
        maps.append(m)
    return maps


def kernel(**inputs) -> np.ndarray:
    nc = _get_program()
    res = run_bass_kernel_spmd(
        nc,
        _in_maps(inputs),
        core_ids=list(range(NCORES)),
        trace=bool(int(os.environ.get("KERNEL_TRACE", "0"))),
    )
    kernel.last_results = res
    # batch order: g-major, fwd core (first 4 batches) then bwd core (last 4)
    outs = []
    for g in range(4):
        outs.append(res.results[g]["out"])
        outs.append(res.results[g + 4]["out"])
    return np.ascontiguousarray(np.concatenate(outs, axis=0))


# revision 20
# speedup vs baseline: 1.1547x; 1.0334x over previous
"""Trainium2 Bass kernel for nn_Luban7_29609504539316 (BiLSTM + span pool + log_softmax).

Sharding (8 cores):
  - Direction-split scan: cores 0-3 run the FORWARD LSTM, cores 4-7 the BACKWARD
    LSTM (fed host-reversed tokens).  Core c handles batch group g = c % 4
    (batches g*8 .. g*8+8) for the scan.
  - Pair (c, c+4) exchanges hidden states (bf16, two time-chunked AllGathers
    issued mid-scan so the collective overlaps the remaining scan steps).
  - Post-LSTM stages are BATCH-SPLIT across the pair: core c handles the first
    4 batches of its group, core c+4 the last 4 (per-core gather-index input).
  - log_softmax over the span axis is single-pass (scores are bounded ~|4.3|):
    local exp-sums are AllReduce-summed over all 8 cores; the log-Z subtraction
    is folded into the output transpose copies.
  - Host concatenates the outputs of all 8 cores in batch order.

The program is identical on all cores (SPMD); direction and batch assignment
live entirely in the per-core input data (tokens, per-direction weights,
gather indices).
"""

import os
import sys

import numpy as np

for _p in ("/opt/trn_rl_repo",):
    if _p not in sys.path and os.path.isdir(_p):
        sys.path.insert(0, _p)

import concourse.bass as bass
import concourse.tile as tile
from concourse import bacc
from concourse import mybir
from concourse.bass_utils import run_bass_kernel_spmd

F32 = mybir.dt.float32
F32R = mybir.dt.float32r
BF16 = mybir.dt.bfloat16
FP8 = mybir.dt.float8e4
I32 = mybir.dt.int32
AF = mybir.ActivationFunctionType
OP = mybir.AluOpType
PM = mybir.MatmulPerfMode
USE_FP8 = bool(int(os.environ.get("KERNEL_FP8", "0")))

# Problem dims (hardcoded per spec)
B, T, V, E, H, C, L = 32, 120, 32000, 256, 512, 20, 10
G4 = 4 * H  # 2048
LN_EPS = 1e-5
NCORES = 8
BL = 8             # batches per scan core
PB = 4             # post-stage batches per core
ROWS = BL * T      # 960
PROWS = PB * T     # 480  (post-stage rows per dir)
CH = 60            # scan steps per AllGather chunk
NCH = T // CH      # 2

# static span table (matches reference loop order)
_begs, _lens = [], []
for _b in range(T):
    for _l in range(1, min(L, T - _b) + 1):
        _begs.append(_b)
        _lens.append(_l)
BEGS = np.asarray(_begs, np.int32)
LENS = np.asarray(_lens, np.int32)
S = len(_begs)  # 1155
assert S == 1155
SROWS = PB * S     # output rows per core = 4620


def _mspanT() -> np.ndarray:
    """[T, S] span-mean pooling matrix (inv_len folded in)."""
    m = np.zeros((T, S), np.float32)
    for s in range(S):
        m[BEGS[s] : BEGS[s] + LENS[s], s] = 1.0 / LENS[s]
    return m


def _gather_idx(is_bwd_core: bool) -> np.ndarray:
    """[2*PROWS, 1] per-core gather table into hs_ag [2*ROWS rows].

    hs_ag row layout (identical on both cores of a pair):
      row = 960*j + 480*r + (s - 60*j)*8 + b_local
    where j = chunk (s//60), r = rank in pair (0=fwd core, 1=bwd core),
    s = scan step, b_local = batch index within the group (0..7).

    Gather order (defines rnnT columns): first fwd rows (pb-major, t-minor),
    then bwd rows.  For the bwd direction scan step s corresponds to time
    t = T-1-s, so we read row for s = T-1-t.
    """
    idx = np.empty(2 * PROWS, np.int32)
    boff = PB if is_bwd_core else 0
    for pb in range(PB):
        bl = boff + pb
        for t in range(T):
            s = t
            j = s // CH
            idx[pb * T + t] = ROWS * j + (s - CH * j) * BL + bl
            s = T - 1 - t
            j = s // CH
            idx[PROWS + pb * T + t] = ROWS * j + PROWS + (s - CH * j) * BL + bl
    return idx[:, None]


def _r(ap):
    return ap.bitcast(F32R)


def build_program():
    max_stage = int(os.environ.get("KERNEL_MAX_STAGE", "99"))
    scan_steps = int(os.environ.get("KERNEL_SCAN_STEPS", str(T)))
    n_warm = int(os.environ.get("KERNEL_WARM", "0"))
    nc = bacc.Bacc(trn_type="TRN2", num_devices=NCORES)

    # ---- I/O ----
    tok = nc.dram_tensor("tok", [ROWS, 1], I32, kind="ExternalInput")
    gidx = nc.dram_tensor("gidx", [2 * PROWS, 1], I32, kind="ExternalInput")
    emb = nc.dram_tensor("emb", [V, E], F32, kind="ExternalInput")
    w_ihT = nc.dram_tensor("w_ihT", [E, G4], F32R, kind="ExternalInput")
    w_hhT = nc.dram_tensor("w_hhT", [H, G4], F32R, kind="ExternalInput")
    w_hhT8 = nc.dram_tensor("w_hhT8", [H, G4], FP8, kind="ExternalInput")
    gbias = nc.dram_tensor("gbias", [1, G4], F32, kind="ExternalInput")
    lin1_wT = nc.dram_tensor("lin1_wT", [2 * H, H], BF16, kind="ExternalInput")
    lin1_b = nc.dram_tensor("lin1_b", [H, 1], F32, kind="ExternalInput")
    lin2_wT = nc.dram_tensor("lin2_wT", [H, H], F32R, kind="ExternalInput")
    lin2_b = nc.dram_tensor("lin2_b", [1, H], F32, kind="ExternalInput")
    ln_g = nc.dram_tensor("ln_g", [1, H], F32, kind="ExternalInput")
    ln_b = nc.dram_tensor("ln_b", [1, H], F32, kind="ExternalInput")
    label_w = nc.dram_tensor("label_w", [H, C], F32R, kind="ExternalInput")
    label_b = nc.dram_tensor("label_b", [C, 1], F32, kind="ExternalInput")
    out = nc.dram_tensor("out", [SROWS, C], F32, kind="ExternalOutput")

    # ---- inline constants (same on every core) ----
    ident_d = nc.inline_tensor(np.eye(128, dtype=np.float32), name="ident")
    mspanT_d = nc.inline_tensor(_mspanT(), name="mspanT")

    with tile.TileContext(nc) as tc:
        # long-lived pools (released at end of build)
        const_p = tc.alloc_tile_pool(name="const", bufs=1)
        whh_p = tc.alloc_tile_pool(name="whh", bufs=1)
        dram_p = tc.alloc_tile_pool(name="dram", bufs=1, space="DRAM")

        ident = const_p.tile([128, 128], F32)
        nc.sync.dma_start(out=ident[:], in_=ident_d[:, :])
        ident_r = const_p.tile([128, 128], F32R)
        nc.gpsimd.dma_start(out=ident_r[:], in_=ident_d[:, :])
        ident_bf = const_p.tile([128, 128], BF16)
        nc.gpsimd.dma_start(out=ident_bf[:], in_=ident_d[:, :])

        if USE_FP8:
            # DoubleRow operand layout: per k-pair j, tile [128, 2*G4] holding
            # the two 128-row k-chunks (2j, 2j+1) adjacent on the free axis.
            whh8 = [whh_p.tile([128, 2 * G4], FP8, tag=f"whh8_{j}", name=f"whh8_{j}") for j in range(2)]
            for j in range(2):
                for i in range(2):
                    nc.sync.dma_start(
                        out=whh8[j][:, G4 * i : G4 * (i + 1)],
                        in_=w_hhT8[128 * (2 * j + i) : 128 * (2 * j + i + 1), :],
                    )
        else:
            w_hhT_sb = [whh_p.tile([128, G4], F32R, tag=f"whh{k}", name=f"whh{k}") for k in range(4)]
            for k in range(4):
                nc.sync.dma_start(out=w_hhT_sb[k][:], in_=w_hhT[128 * k : 128 * (k + 1), :])

        xg_dram = dram_p.tile([T, BL, G4], F32R)
        # per-chunk hs (bf16, t-major) + AllGather output
        hs_dram = [dram_p.tile([CH * BL, H], BF16, name=f"hsd{j}") for j in range(NCH)]
        hs_ag = dram_p.tile([2 * ROWS, H], BF16)

        # ================= Stage 1: embedding gather + transpose =============
        with tc.tile_pool(name="s1", bufs=3) as s1p, \
             tc.tile_pool(name="s1ps", bufs=4, space="PSUM") as s1ps, \
             tc.tile_pool(name="xt", bufs=1) as xt_p, \
             tc.tile_pool(name="wih", bufs=1) as wih_p:

            xT = [xt_p.tile([128, ROWS], F32R, tag=f"xT{k}", name=f"xT{k}") for k in range(2)]
            w_ihT_sb = [wih_p.tile([128, G4], F32R, tag=f"wih{k}", name=f"wih{k}") for k in range(2)]
            gbias_sb = wih_p.tile([T, G4], F32)
            for k in range(2):
                nc.sync.dma_start(
                    out=w_ihT_sb[k][:], in_=w_ihT[128 * k : 128 * (k + 1), :]
                )
            nc.gpsimd.dma_start(
                out=gbias_sb[:], in_=gbias[:, :].to_broadcast([T, G4])
            )

            n_full = ROWS // 128          # 7
            tail = ROWS - n_full * 128    # 64
            for r in range(n_full + 1):
                rows = 128 if r < n_full else tail
                idx_sb = s1p.tile([128, 1], I32, tag="idx")
                x_sb = s1p.tile([128, E], F32, tag="x")
                nc.sync.dma_start(
                    out=idx_sb[:rows], in_=tok[r * 128 : r * 128 + rows, :]
                )
                nc.gpsimd.indirect_dma_start(
                    out=x_sb[:rows, :],
                    out_offset=None,
                    in_=emb[:, :],
                    in_offset=bass.IndirectOffsetOnAxis(ap=idx_sb[:rows, :1], axis=0),
                )
                for k in range(2):
                    pt = s1ps.tile([128, 128], F32, tag="pt")
                    nc.tensor.transpose(
                        out=pt[:, :rows],
                        in_=x_sb[:rows, 128 * k : 128 * (k + 1)],
                        identity=ident[:rows, :rows],
                    )
                    nc.vector.tensor_copy(
                        out=xT[k][:, r * 128 : r * 128 + rows], in_=pt[:, :rows]
                    )

            # ================= Stage 2: xg = x @ w_ihT + bias ================
            with tc.tile_pool(name="s2", bufs=3) as s2p, \
                 tc.tile_pool(name="s2ps", bufs=3, space="PSUM") as s2ps:
                for b in range(BL):
                    for n in range(4):
                        ps = s2ps.tile([T, 512], F32, tag="ps")
                        for k in range(2):
                            nc.tensor.matmul(
                                ps[:],
                                lhsT=xT[k][:, b * T : (b + 1) * T],
                                rhs=w_ihT_sb[k][:, 512 * n : 512 * (n + 1)],
                                start=(k == 0),
                                stop=(k == 1),
                            )
                        stg = s2p.tile([T, 512], F32R, tag="stg")
                        nc.vector.tensor_tensor(
                            out=stg[:],
                            in0=ps[:],
                            in1=gbias_sb[:, 512 * n : 512 * (n + 1)],
                            op=OP.add,
                        )
                        nc.sync.dma_start(
                            out=xg_dram[:, b, 512 * n : 512 * (n + 1)], in_=stg[:]
                        )

        # ================= Stage 3: LSTM scan (this core's direction) ========
        # Layout: one PSUM tile per gate pg[32,512]; batch padded 8->32 with
        # zeros so every read row is defined.  The xg contribution is injected
        # by an identity-matmul per gate.  h lives in h_pad [32,512]; one
        # transpose chain produces hT_all [128,128] (f32r) used as the next
        # step's stationary operand.  hs is stored bf16 (cast during DMA) into
        # per-chunk DRAM tiles; each chunk AllGathers with the pair core as
        # soon as its last step is stored, overlapping the remaining scan.
        with tc.tile_pool(name="state", bufs=1) as st_p, \
             tc.tile_pool(name="xg", bufs=3) as xg_p, \
             tc.tile_pool(name="gt", bufs=3) as gt_p, \
             tc.tile_pool(name="gps", bufs=2, space="PSUM") as gps, \
             tc.tile_pool(name="tps", bufs=2, space="PSUM") as tps, \
             tc.tile_pool(name="dps", bufs=1, space="PSUM") as dps:

            c_sb = st_p.tile([BL, H], F32)
            nc.vector.memset(c_sb[:], 0.0)
            # h_pad rotates through several buffers so the hs-store DMA (and
            # the AllGather blocking the gpsimd queue mid-scan) never stalls
            # the next step's h write.
            NHP = 6
            h_pads = [st_p.tile([32, H], F32, tag=f"hp{i}", name=f"hp{i}") for i in range(NHP)]
            for i in range(NHP):
                nc.vector.memset(h_pads[i][:], 0.0)
            hT_all = st_p.tile([128, 128], FP8 if USE_FP8 else F32R)
            if USE_FP8:
                nc.vector.memset(hT_all[:], 0.0)
            else:
                nc.vector.memset(hT_all[:].bitcast(F32), 0.0)

            pdum = dps.tile([32, 512], F32, name="pdum")

            # gate order: gg, i, f, o — the tanh chain starts as early as possible
            GATE_ORDER = (2, 0, 1, 3)
            for s in range(scan_steps if max_stage >= 3 else 1):
                h_pad = h_pads[s % NHP]
                xg_s = xg_p.tile([BL, G4], F32R, tag="xg")
                nc.sync.dma_start(out=xg_s[:], in_=xg_dram[s, :, :])

                acts = {}
                for gi, n in enumerate(GATE_ORDER):
                    pg = gps.tile([32, 512], F32, tag=f"pg{n}", bufs=1, name=f"pg{n}")
                    nc.tensor.matmul(
                        pg[:],
                        lhsT=ident_r[:BL, :32],
                        rhs=xg_s[:, 512 * n : 512 * (n + 1)],
                        start=True,
                        stop=False,
                    )
                    if USE_FP8:
                        for j in range(2):
                            nc.tensor.matmul(
                                pg[:],
                                lhsT=hT_all[:, 64 * j : 64 * (j + 1)].rearrange(
                                    "p (i b) -> p i b", i=2
                                ),
                                rhs=whh8[j][:, :].rearrange(
                                    "p (i c) -> p i c", i=2
                                )[:, :, 512 * n : 512 * (n + 1)],
                                start=False,
                                stop=(j == 1),
                                perf_mode=PM.DoubleRow,
                            )
                    else:
                        for k in range(4):
                            nc.tensor.matmul(
                                pg[:],
                                lhsT=hT_all[:, 32 * k : 32 * (k + 1)],
                                rhs=w_hhT_sb[k][:, 512 * n : 512 * (n + 1)],
                                start=False,
                                stop=(k == 3),
                            )
                    a_sb = gt_p.tile([BL, 512], F32, tag=f"a{n}")
                    if n == 1:  # f: halved so the c-chain pipelines
                        for hf in (0, 1):
                            sl = slice(256 * hf, 256 * (hf + 1))
                            nc.scalar.activation(
                                out=a_sb[:, sl], in_=pg[:BL, sl], func=AF.Sigmoid
                            )
                    else:
                        nc.scalar.activation(
                            out=a_sb[:],
                            in_=pg[:BL, :],
                            func=AF.Tanh if n == 2 else AF.Sigmoid,
                        )
                    acts[n] = a_sb
                    if n == 0:  # after i (2nd group): t1 = sig_i * tanh_gg
                        t1 = gt_p.tile([BL, H], F32, tag="t1")
                        nc.vector.tensor_tensor(
                            out=t1[:], in0=acts[0][:], in1=acts[2][:], op=OP.mult
                        )
                    elif n == 1:  # after f (3rd group): c = c*f + t1; tanh(c)
                        tch = gt_p.tile([BL, H], F32, tag="tch")
                        for hf in (0, 1):
                            sl = slice(256 * hf, 256 * (hf + 1))
                            nc.vector.tensor_tensor(
                                out=c_sb[:, sl], in0=c_sb[:, sl], in1=acts[1][:, sl],
                                op=OP.mult,
                            )
                            nc.vector.tensor_tensor(
                                out=c_sb[:, sl], in0=c_sb[:, sl], in1=t1[:, sl],
                                op=OP.add,
                            )
                            nc.scalar.activation(
                                out=tch[:, sl], in_=c_sb[:, sl], func=AF.Tanh
                            )

                # optional PE warming filler
                for _ in range(n_warm):
                    nc.tensor.matmul(
                        pdum[:],
                        lhsT=ident_r[:BL, :32],
                        rhs=xg_s[:, :512],
                        start=True,
                        stop=True,
                    )

                # h = sig_o * tanh(c)
                for hf in (0, 1):
                    sl = slice(256 * hf, 256 * (hf + 1))
                    nc.vector.tensor_tensor(
                        out=h_pad[:BL, sl], in0=acts[3][:, sl], in1=tch[:, sl],
                        op=OP.mult,
                    )
                nc.gpsimd.dma_start(
                    out=hs_dram[s // CH][(s % CH) * BL : (s % CH + 1) * BL, :],
                    in_=h_pad[:BL, :],
                )

                pt_all = tps.tile([128, 128], F32, tag="pt")
                for q in range(4):
                    nc.tensor.transpose(
                        out=pt_all[:, 32 * q : 32 * (q + 1)],
                        in_=h_pad[:, 128 * q : 128 * (q + 1)],
                        identity=ident[:32, :32],
                    )
                nc.vector.tensor_copy(out=hT_all[:], in_=pt_all[:])

                # ====== Stage 4: chunked AllGather with the pair core =======
                if max_stage >= 4 and (s + 1) % CH == 0 and s + 1 <= NCH * CH:
                    j = (s + 1) // CH - 1
                    nc.gpsimd.collective_compute(
                        "AllGather",
                        OP.bypass,
                        replica_groups=[[0, 4], [1, 5], [2, 6], [3, 7]],
                        ins=[hs_dram[j][:].opt()],
                        outs=[hs_ag[ROWS * j : ROWS * (j + 1), :].opt()],
                    )

        whh_p.release()

        # ================= Stage 5: reorder-gather + transpose -> rnnT =======
        with tc.tile_pool(name="h1T", bufs=1) as h1_p, \
             tc.tile_pool(name="scT", bufs=1) as sc_p:

            h1T = [h1_p.tile([128, PROWS], F32R, tag=f"h1T{m}", name=f"h1T{m}") for m in range(4)]
            scoresT = sc_p.tile([C, SROWS], F32)

            with tc.tile_pool(name="rnnT", bufs=1) as rt_p:
                rnnT = [rt_p.tile([128, PROWS], BF16, tag=f"rnnT{j}", name=f"rnnT{j}") for j in range(8)]

                with tc.tile_pool(name="s5", bufs=3) as s5p, \
                     tc.tile_pool(name="s5ps", bufs=4, space="PSUM") as s5ps:
                    n_ch = (2 * PROWS + 127) // 128  # 8 (last chunk 64 rows)
                    for cch in range(n_ch if max_stage >= 5 else 0):
                        rows = min(128, 2 * PROWS - cch * 128)
                        idx_sb = s5p.tile([128, 1], I32, tag="gidx")
                        nc.sync.dma_start(
                            out=idx_sb[:rows],
                            in_=gidx[cch * 128 : cch * 128 + rows, :],
                        )
                        t_sb = s5p.tile([128, H], BF16, tag="hrows")
                        nc.gpsimd.indirect_dma_start(
                            out=t_sb[:rows, :],
                            out_offset=None,
                            in_=hs_ag[:, :],
                            in_offset=bass.IndirectOffsetOnAxis(ap=idx_sb[:rows, :1], axis=0),
                        )
                        g0 = cch * 128  # global gathered row
                        for k in range(4):
                            pt = s5ps.tile([128, 128], BF16, tag="pt")
                            nc.tensor.transpose(
                                out=pt[:, :rows], in_=t_sb[:rows, 128 * k : 128 * (k + 1)],
                                identity=ident_bf[:rows, :rows],
                            )
                            # rows g0..g0+rows map to (dir = g//PROWS, col = g%PROWS)
                            if (g0 // PROWS) == ((g0 + rows - 1) // PROWS):
                                dd = g0 // PROWS
                                nc.vector.tensor_copy(
                                    out=rnnT[dd * 4 + k][:, g0 % PROWS : g0 % PROWS + rows],
                                    in_=pt[:, :rows],
                                )
                            else:  # chunk straddles the fwd/bwd boundary
                                n0 = PROWS - g0
                                nc.vector.tensor_copy(
                                    out=rnnT[k][:, g0 : g0 + n0], in_=pt[:, :n0]
                                )
                                nc.vector.tensor_copy(
                                    out=rnnT[4 + k][:, 0 : rows - n0], in_=pt[:, n0:rows]
                                )

                # ================= Stage 6: h1T = relu(lin1) ====================
                with tc.tile_pool(name="s6w", bufs=1) as s6w, \
                     tc.tile_pool(name="s6ps", bufs=2, space="PSUM") as s6ps:

                    l1w = [s6w.tile([128, H], BF16, tag=f"l1w{k}", name=f"l1w{k}") for k in range(8)]
                    for k in range(8):
                        nc.sync.dma_start(
                            out=l1w[k][:], in_=lin1_wT[128 * k : 128 * (k + 1), :]
                        )
                    l1b = [s6w.tile([128, 1], F32, tag=f"l1b{m}", name=f"l1b{m}") for m in range(4)]
                    for m in range(4):
                        nc.sync.dma_start(
                            out=l1b[m][:], in_=lin1_b[128 * m : 128 * (m + 1), :]
                        )

                    for m in range(4 if max_stage >= 6 else 0):
                        ph = s6ps.tile([128, PROWS], F32, tag="ph")
                        for k in range(8):
                            nc.tensor.matmul(
                                ph[:],
                                lhsT=l1w[k][:, 128 * m : 128 * (m + 1)],
                                rhs=rnnT[k][:],
                                start=(k == 0),
                                stop=(k == 7),
                            )
                        nc.scalar.activation(
                            out=h1T[m][:],
                            in_=ph[:],
                            func=AF.Relu,
                            bias=l1b[m][:],
                        )

            # ============ Stages 7-10: per-batch lin2+LN+span+label ==========
            with tc.tile_pool(name="s7w", bufs=1) as s7w, \
                 tc.tile_pool(name="s7", bufs=3) as s7p, \
                 tc.tile_pool(name="spT", bufs=1) as sp_p, \
                 tc.tile_pool(name="s7ps", bufs=2, space="PSUM") as s7ps:

                l2w = [s7w.tile([128, H], F32R, tag=f"l2w{k}", name=f"l2w{k}") for k in range(4)]
                for k in range(4):
                    nc.sync.dma_start(
                        out=l2w[k][:], in_=lin2_wT[128 * k : 128 * (k + 1), :]
                    )
                l2b = s7w.tile([T, H], F32)
                nc.gpsimd.dma_start(out=l2b[:], in_=lin2_b[:, :].to_broadcast([T, H]))
                lng = s7w.tile([T, H], F32)
                nc.gpsimd.dma_start(out=lng[:], in_=ln_g[:, :].to_broadcast([T, H]))
                lnb = s7w.tile([T, H], F32)
                nc.gpsimd.dma_start(out=lnb[:], in_=ln_b[:, :].to_broadcast([T, H]))
                lblw = [s7w.tile([128, C], F32R, tag=f"lblw{k}", name=f"lblw{k}") for k in range(4)]
                for k in range(4):
                    nc.sync.dma_start(
                        out=lblw[k][:], in_=label_w[128 * k : 128 * (k + 1), :]
                    )
                lblb = s7w.tile([C, 1], F32)
                nc.sync.dma_start(out=lblb[:], in_=label_b[:, :])
                SP = 1160  # S padded (fp32r matmul needs even free dims)
                mspan = s7w.tile([T, SP], F32R)
                nc.vector.memset(mspan[:].bitcast(F32), 0.0)
                nc.gpsimd.dma_start(out=mspan[:, :S], in_=mspanT_d[:, :])
                eps_sb = s7w.tile([T, 1], F32)
                nc.vector.memset(eps_sb[:], LN_EPS)
                partials = s7w.tile([C, PB], F32)

                SCH = ((0, 512), (512, 512), (1024, 136))
                for b in range(PB if max_stage >= 7 else 0):
                    ph2 = s7ps.tile([T, 512], F32, tag="ph2")
                    for k in range(4):
                        nc.tensor.matmul(
                            ph2[:],
                            lhsT=h1T[k][:, b * T : (b + 1) * T],
                            rhs=l2w[k][:],
                            start=(k == 0),
                            stop=(k == 3),
                        )
                    tr = s7p.tile([T, H], F32R, tag="tr")
                    nc.vector.tensor_tensor(
                        out=tr[:], in0=ph2[:],
                        in1=l2b[:], op=OP.add,
                    )
                    # LayerNorm over H
                    stats = s7p.tile([T, 6], F32, tag="stats")
                    nc.vector.bn_stats(out=stats[:], in_=tr[:])
                    mv = s7p.tile([T, 2], F32, tag="mv")
                    nc.vector.bn_aggr(out=mv[:], in_=stats[:])
                    sd = s7p.tile([T, 1], F32, tag="sd")
                    nc.scalar.activation(
                        out=sd[:], in_=mv[:, 1:2], func=AF.Sqrt, bias=eps_sb[:]
                    )
                    rstd = s7p.tile([T, 1], F32, tag="rstd")
                    nc.vector.reciprocal(out=rstd[:], in_=sd[:])
                    nc.vector.tensor_scalar(
                        out=tr[:], in0=tr[:],
                        scalar1=mv[:, 0:1], scalar2=rstd[:],
                        op0=OP.subtract, op1=OP.mult,
                    )
                    nc.vector.tensor_tensor(
                        out=tr[:], in0=tr[:],
                        in1=lng[:], op=OP.mult,
                    )
                    nc.vector.tensor_tensor(
                        out=tr[:], in0=tr[:],
                        in1=lnb[:], op=OP.add,
                    )
                    # span pooling (transposed): spansT[m] = tr[:,m-chunk].T @ mspanT
                    spansT = [
                        sp_p.tile([128, SP], F32R, tag=f"spansT{m}", name=f"spansT{m}")
                        for m in range(4)
                    ]
                    for m in range(4):
                        for ci, (n0, nw) in enumerate(SCH):
                            psp = s7ps.tile([128, 512], F32, tag="psp")
                            nc.tensor.matmul(
                                psp[:, :nw],
                                lhsT=tr[:, 128 * m : 128 * (m + 1)],
                                rhs=mspan[:, n0 : n0 + nw],
                                start=True,
                                stop=True,
                            )
                            if (m * 3 + ci) % 2 == 0:
                                nc.vector.tensor_copy(
                                    out=spansT[m][:, n0 : n0 + nw], in_=psp[:, :nw]
                                )
                            else:
                                nc.scalar.copy(
                                    out=spansT[m][:, n0 : n0 + nw], in_=psp[:, :nw]
                                )
                    # label scores (transposed): scoresT = label_w.T @ spansT
                    for n0, nw in SCH:
                        psc = s7ps.tile([C, 512], F32, tag="psc")
                        for k in range(4):
                            nc.tensor.matmul(
                                psc[:, :nw],
                                lhsT=lblw[k][:],
                                rhs=spansT[k][:, n0 : n0 + nw],
                                start=(k == 0),
                                stop=(k == 3),
                            )
                        w_real = min(nw, S - n0)
                        nc.scalar.activation(
                            out=scoresT[:, b * S + n0 : b * S + n0 + w_real],
                            in_=psc[:, :w_real],
                            func=AF.Identity,
                            bias=lblb[:],
                        )
                    # single-pass softmax: exp-sum of this batch's scores
                    expc = s7p.tile([C, S], F32, tag="expc")
                    nc.scalar.activation(
                        out=expc[:], in_=scoresT[:, b * S : (b + 1) * S],
                        func=AF.Exp,
                        accum_out=partials[:, b : b + 1],
                    )

                # ============ Stage 10: global log-sum-exp ===============
                cc_in = dram_p.tile([C, 32], F32)
                cc_out = dram_p.tile([C, 32], F32, addr_space="Shared")
                RG = [[0, 1, 2, 3, 4, 5, 6, 7]]

                if max_stage < 10:
                    nc.vector.memset(scoresT[:, :4], 0.0)  # keep scoresT written
                lsum = s7p.tile([C, 1], F32, tag="lsum")
                nc.vector.tensor_reduce(
                    out=lsum[:], in_=partials[:], axis=mybir.AxisListType.X,
                    op=OP.add,
                )
                stg32 = s7p.tile([C, 32], F32, tag="stg32")
                nc.vector.tensor_copy(out=stg32[:], in_=lsum[:].to_broadcast([C, 32]))
                nc.sync.dma_start(out=cc_in[:, :], in_=stg32[:])
                gsum = s7p.tile([C, 32], F32, tag="gsum")
                if max_stage >= 10:
                    nc.gpsimd.collective_compute(
                        "AllReduce", OP.add, replica_groups=RG,
                        ins=[cc_in[:].opt()], outs=[cc_out[:].opt()],
                    )
                    nc.sync.dma_start(out=gsum[:], in_=cc_out[:, :])
                else:
                    nc.vector.tensor_copy(out=gsum[:], in_=stg32[:])
                logz = s7p.tile([C, 1], F32, tag="logz")
                nc.scalar.activation(out=logz[:], in_=gsum[:, 0:1], func=AF.Ln)
                normT = scoresT  # in place
                nc.vector.tensor_scalar(
                    out=normT[:], in0=scoresT[:], scalar1=logz[:], scalar2=None,
                    op0=OP.subtract,
                )

            # ============ Stage 11: transpose out ====================
            with tc.tile_pool(name="outp", bufs=1) as out_p, \
                 tc.tile_pool(name="ops", bufs=4, space="PSUM") as ops:
                n_full = SROWS // 128  # 36
                tail = SROWS - n_full * 128  # 12
                out_sb = out_p.tile([128, n_full * C], F32)
                out_tail = out_p.tile([tail, C], F32)
                for m in range(n_full + 1):
                    cw = 128 if m < n_full else tail
                    pto = ops.tile([128, C], F32, tag="pto")
                    nc.tensor.transpose(
                        out=pto[:cw, :],
                        in_=normT[:, 128 * m : 128 * m + cw],
                        identity=ident[:C, :C],
                    )
                    if m < n_full:
                        if m % 2 == 0:
                            nc.vector.tensor_copy(
                                out=out_sb[:, m * C : (m + 1) * C], in_=pto[:, :]
                            )
                        else:
                            nc.scalar.copy(
                                out=out_sb[:, m * C : (m + 1) * C], in_=pto[:, :]
                            )
                    else:
                        nc.vector.tensor_copy(
                            out=out_tail[:], in_=pto[:cw, :]
                        )
                out_v = out[: n_full * 128, :].rearrange(
                    "(m p) c -> p m c", p=128
                )
                nc.sync.dma_start(out=out_v, in_=out_sb[:].rearrange(
                    "p (m c) -> p m c", c=C))
                nc.sync.dma_start(
                    out=out[n_full * 128 :, :], in_=out_tail[:]
                )

        const_p.release()
        dram_p.release()

    nc.finalize()
    return nc


_PROGRAM = None


def _get_program():
    global _PROGRAM
    if _PROGRAM is None:
        _PROGRAM = build_program()
    return _PROGRAM


def _in_maps(inputs: dict) -> list[dict]:
    import ml_dtypes

    tokens = np.ascontiguousarray(np.asarray(inputs["tokens"], np.int32))
    emb = np.ascontiguousarray(np.asarray(inputs["emb"], np.float32))
    shared = {
        "emb": emb,
        "lin1_wT": np.ascontiguousarray(
            np.asarray(inputs["lin1_w"], np.float32).T.astype(ml_dtypes.bfloat16)
        ),
        "lin1_b": np.ascontiguousarray(
            np.asarray(inputs["lin1_b"], np.float32)[:, None]
        ),
        "lin2_wT": np.ascontiguousarray(np.asarray(inputs["lin2_w"], np.float32).T),
        "lin2_b": np.ascontiguousarray(
            np.asarray(inputs["lin2_b"], np.float32)[None, :]
        ),
        "ln_g": np.ascontiguousarray(np.asarray(inputs["ln_g"], np.float32)[None, :]),
        "ln_b": np.ascontiguousarray(np.asarray(inputs["ln_b"], np.float32)[None, :]),
        "label_w": np.ascontiguousarray(np.asarray(inputs["label_w"], np.float32)),
        "label_b": np.ascontiguousarray(
            np.asarray(inputs["label_b"], np.float32)[:, None]
        ),
    }
    fp8_np = mybir.dt.np(FP8)
    per_dir = {}
    for d, sfx in ((0, "f"), (1, "b")):
        whht = np.asarray(inputs[f"w_hh_{sfx}"], np.float32).T
        per_dir[d] = {
            "w_ihT": np.ascontiguousarray(
                np.asarray(inputs[f"w_ih_{sfx}"], np.float32).T
            ),
            "w_hhT": np.ascontiguousarray(whht),
            "w_hhT8": np.ascontiguousarray(whht.astype(fp8_np)),
            "gbias": np.ascontiguousarray(
                (
                    np.asarray(inputs[f"b_ih_{sfx}"], np.float32)
                    + np.asarray(inputs[f"b_hh_{sfx}"], np.float32)
                )[None, :]
            ),
        }
    gidx_f = _gather_idx(False)
    gidx_b = _gather_idx(True)
    maps = []
    for core in range(NCORES):
        d = core // 4
        g = core % 4
        tk = tokens[g * BL : (g + 1) * BL]  # [8, 120]
        if d == 1:
            tk = tk[:, ::-1]
        m = dict(shared)
        m.update(per_dir[d])
        m["tok"] = np.ascontiguousarray(tk.reshape(-1)[:, None])
        m["gidx"] = np.ascontiguousarray(gidx_b if d == 1 else gidx_f)
        maps.append(m)
    return maps


def kernel(**inputs) -> np.ndarray:
    nc = _get_program()
    res = run_bass_kernel_spmd(
        nc,
        _in_maps(inputs),
        core_ids=list(range(NCORES)),
        trace=bool(int(os.environ.get("KERNEL_TRACE", "0"))),
    )
    kernel.last_results = res
    # batch order: g-major, fwd core (first 4 batches) then bwd core (last 4)
    outs = []
    for g in range(4):
        outs.append(res.results[g]["out"])
        outs.append(res.results[g + 4]["out"])
    return np.ascontiguousarray(np.concatenate(outs, axis=0))
